# revision 13
# baseline (speedup 1.0000x reference)
"""Fused LayerNorm + 16-head self-attention + output projection on 8 NeuronCores.

Sharding: core c = (batch b = c//2, head-group g = c%2).  Data parallel over
the 4 batches; tensor parallel over head groups (8 heads each, Megatron-style
column split of W_q/W_kv and row split of W_out).  The two partial outputs
per batch are summed on the host.

Per-core pipeline (all matmuls bf16; fp8 was tried and rejected - exp in
fp8 alone costs 2.2e-2 rel err vs the 2e-2 gate):
  A: LayerNorm stats on DVE, apply on GpSimd (xn bf16), PE transposes
     (bf16) -> xnt [128, 8, 2048]; v projection interleaved.
  B: qT/kT projections per head pair (weight chunks stationary, xnt moving).
  C: attention per (query half, head pair), one head at a time: S^T = kT.T
     @ qT (K=64); exp on ACT (the hard ~280us/core floor) -> bf16 e tiles;
     O^T = vaug.T @ e with a ones column at col 64 producing the softmax
     denominator at psum row 64.  O runs 4 kc behind exp so the PE never
     waits on the activation engine.  Normalize: DVE reciprocal + gpsimd
     partition_broadcast + DVE mul -> attnt bf16.
  D: out = attnt.T @ W_out, streamed to DRAM.

The emission is one long software pipeline tuned to keep the PE (the
bottleneck at ~400us busy) dense: all projection/outproj work is broken
into 1-2-matmul "filler steps" that are drained into the ACT-bound
attention slots at a per-slot rate, with the PSUM split as: 2x S tiles
(4 banks), 2x O accumulators [65,512] (2 banks), 1 filler accumulator
(2 banks).
"""

import math

import numpy as np
import ml_dtypes

import concourse.bacc as bacc
import concourse.tile as tile
from concourse import mybir
from concourse.bass_utils import run_bass_kernel_spmd
from concourse.masks import make_identity

F32 = mybir.dt.float32
BF16 = mybir.dt.bfloat16

B, N, D = 4, 2048, 1024
H_TOT, DH, E = 16, 64, 1024
NCORES = 8
HL = 8            # heads per core
EL = HL * DH      # 512 local embed
NT = N // 128     # 16 token tiles
NDC = D // 128    # 8 contraction chunks
NP = 4            # head pairs per core
SCALE = float(DH) ** -0.5
EPS = 1e-5
ODELAY = 4        # O-matmul lag (in kc) behind its exp

_nc_cache = {}


def _build_nc():
    nc = bacc.Bacc("TRN2", target_bir_lowering=False)
    x = nc.dram_tensor("x", [N, D], BF16, kind="ExternalInput").ap()
    wq = nc.dram_tensor("wq", [D, EL], BF16, kind="ExternalInput").ap()
    wk = nc.dram_tensor("wk", [D, EL], BF16, kind="ExternalInput").ap()
    wv = nc.dram_tensor("wv", [D, EL], BF16, kind="ExternalInput").ap()
    wo = nc.dram_tensor("wo", [EL, D], BF16, kind="ExternalInput").ap()
    out = nc.dram_tensor("out", [N, D], F32, kind="ExternalOutput").ap()

    with tile.TileContext(nc) as tc:
        with (
            tc.tile_pool(name="consts", bufs=1) as consts,
            tc.tile_pool(name="bigsb", bufs=1) as bigsb,
            tc.tile_pool(name="xload", bufs=3) as xload,
            tc.tile_pool(name="xnp", bufs=3) as xnp,
            tc.tile_pool(name="stats", bufs=8) as stats,
            tc.tile_pool(name="wsmall", bufs=16) as wsmall,
            tc.tile_pool(name="e2p", bufs=8) as e2p,
            tc.tile_pool(name="small", bufs=3) as small,
            tc.tile_pool(name="osb", bufs=2) as osbp,
            tc.tile_pool(name="pbig", bufs=2, space="PSUM") as pbig,
            tc.tile_pool(name="poacc", bufs=2, space="PSUM") as poacc,
            tc.tile_pool(name="pfill", bufs=1, space="PSUM") as pfill,
        ):
            ident = consts.tile([128, 128], BF16, tag="ident", name="ident")
            make_identity(nc, ident)
            eps_t = consts.tile([128, 1], F32, tag="eps", name="eps")
            nc.vector.memset(eps_t, EPS)

            xnt = bigsb.tile([128, NDC, N], BF16, tag="xnt", name="xnt")
            qt = [
                bigsb.tile([128, N], BF16, tag=f"qt{p}", name=f"qt{p}")
                for p in range(NP)
            ]
            kt = [
                bigsb.tile([128, N], BF16, tag=f"kt{p}", name=f"kt{p}")
                for p in range(NP)
            ]
            attnt = [
                bigsb.tile([128, N], BF16, tag=f"at{p}", name=f"at{p}")
                for p in range(NP)
            ]
            # vaug[:, m, h, 0:64]=v, [.., 64]=1 (ones col -> denominator)
            vaug = bigsb.tile([128, NT, HL, 65], BF16, tag="vaug", name="vaug")
            nc.vector.memset(vaug[:, :, :, 64:65], 1.0)

            wvsb = bigsb.tile([128, NDC, EL], BF16, tag="wvsb", name="wvsb")
            for d in range(NDC):
                nc.sync.dma_start(
                    out=wvsb[:, d, :], in_=wv[d * 128 : (d + 1) * 128, :]
                )
            wosb = bigsb.tile([128, NP, D], BF16, tag="wosb", name="wosb")
            for ec in range(NP):
                nc.sync.dma_start(
                    out=wosb[:, ec, :], in_=wo[ec * 128 : (ec + 1) * 128, :]
                )

            # ---------------- LN + transpose ----------------------------
            def ln_stats(m, nsplit=2):
                xt = xload.tile([128, D], BF16, tag="xt", name="xt")
                # column-chunk DMAs land on different queues: lower
                # first-tile latency (each queue is descriptor-rate-bound)
                w = D // nsplit
                for cc in range(nsplit):
                    nc.sync.dma_start(
                        out=xt[:, cc * w : (cc + 1) * w],
                        in_=x[m * 128 : (m + 1) * 128, cc * w : (cc + 1) * w],
                    )
                st = stats.tile([128, 2, 6], F32, tag="bn", name="bn")
                nc.vector.bn_stats(out=st[:, 0, :], in_=xt[:, 0:512])
                nc.vector.bn_stats(out=st[:, 1, :], in_=xt[:, 512:1024])
                mv = stats.tile([128, 2], F32, tag="mv", name="mv")
                nc.vector.bn_aggr(out=mv, in_=st)
                sq = stats.tile([128, 1], F32, tag="sq", name="sq")
                nc.scalar.activation(
                    out=sq,
                    in_=mv[:, 1:2],
                    func=mybir.ActivationFunctionType.Sqrt,
                    bias=eps_t,
                    scale=1.0,
                )
                rec = stats.tile([128, 1], F32, tag="rec", name="rec")
                nc.vector.reciprocal(out=rec, in_=sq)
                nmr = stats.tile([128, 1], F32, tag="nmr", name="nmr")
                nc.vector.tensor_scalar(
                    out=nmr,
                    in0=mv[:, 0:1],
                    scalar1=rec,
                    scalar2=-1.0,
                    op0=mybir.AluOpType.mult,
                    op1=mybir.AluOpType.mult,
                )
                xn = xnp.tile([128, D], BF16, tag="xn", name="xn")
                nc.gpsimd.tensor_scalar(
                    out=xn,
                    in0=xt,
                    scalar1=rec,
                    scalar2=nmr,
                    op0=mybir.AluOpType.mult,
                    op1=mybir.AluOpType.add,
                )
                return xn

            def ln_transpose(m, xn):
                for dp in range(NDC // 2):
                    trp = pbig.tile([128, 2, 128], BF16, tag="big", name="trp")
                    for j in range(2):
                        d = 2 * dp + j
                        nc.tensor.transpose(
                            trp[:, j, :], xn[:, d * 128 : (d + 1) * 128], ident
                        )
                    nc.scalar.copy(
                        out=xnt[:, 2 * dp : 2 * dp + 2, m * 128 : (m + 1) * 128],
                        in_=trp,
                    )

            # ---------------- filler step generators --------------------
            # Each filler is a list of closures emitting ~1-2 matmuls (or a
            # copy); consecutive steps of one unit share a pfill psum tile.

            def qk_quarter_steps(p, w_dram, dst, half):
                """q/k projection quarter as 9 fine-grained steps."""
                state = {}

                def start():
                    state["wts"] = []
                    for d in range(NDC):
                        wt = wsmall.tile([128, 128], BF16, tag="w", name="w")
                        nc.sync.dma_start(
                            out=wt,
                            in_=w_dram[
                                d * 128 : (d + 1) * 128, p * 128 : (p + 1) * 128
                            ],
                        )
                        state["wts"].append(wt)
                    state["pt"] = pfill.tile([128, 1024], F32, tag="f", name="ptq")

                def mm(d):
                    for ns in range(2):
                        nc.tensor.matmul(
                            out=state["pt"][:, ns * 512 : (ns + 1) * 512],
                            lhsT=state["wts"][d],
                            rhs=xnt[
                                :,
                                d,
                                half * 1024 + ns * 512 : half * 1024 + (ns + 1) * 512,
                            ],
                            start=(d == 0),
                            stop=(d == NDC - 1),
                        )

                def fin():
                    nc.vector.tensor_copy(
                        out=dst[:, half * 1024 : (half + 1) * 1024],
                        in_=state["pt"],
                    )

                def first():
                    start()
                    mm(0)

                return [first] + [
                    (lambda d=d: mm(d)) for d in range(1, NDC)
                ] + [fin]

            def v_steps(m):
                """v projection for token tile m as 5 steps."""
                state = {}

                def mm(d0):
                    if d0 == 0:
                        state["pv"] = pfill.tile(
                            [128, EL], F32, tag="f", name="pv"
                        )
                    for d in (d0, d0 + 1):
                        nc.tensor.matmul(
                            out=state["pv"],
                            lhsT=xnt[:, d, m * 128 : (m + 1) * 128],
                            rhs=wvsb[:, d, :],
                            start=(d == 0),
                            stop=(d == NDC - 1),
                        )

                def fin():
                    nc.scalar.copy(
                        out=vaug[:, m, :, 0:64],
                        in_=state["pv"].rearrange("p (h dh) -> p h dh", h=HL),
                    )

                return [(lambda d0=d0: mm(d0)) for d0 in range(0, NDC, 2)] + [fin]

            def outproj_steps(m):
                """output projection for token tile m as 5 steps."""
                state = {}

                def mm(ec):
                    if ec == 0:
                        state["pt"] = pfill.tile(
                            [128, 1024], F32, tag="f", name="pto"
                        )
                    for ns in range(2):
                        nc.tensor.matmul(
                            out=state["pt"][:, ns * 512 : (ns + 1) * 512],
                            lhsT=attnt[ec][:, m * 128 : (m + 1) * 128],
                            rhs=wosb[:, ec, ns * 512 : (ns + 1) * 512],
                            start=(ec == 0),
                            stop=(ec == NP - 1),
                        )

                def fin():
                    ob = osbp.tile([128, D], F32, tag="ob", name="ob")
                    nc.vector.tensor_copy(out=ob, in_=state["pt"])
                    nc.sync.dma_start(
                        out=out[m * 128 : (m + 1) * 128, :], in_=ob
                    )

                return [(lambda ec=ec: mm(ec)) for ec in range(NP)] + [fin]

            # ---------------- attention ---------------------------------
            def attention_block(p, qh, fillers, fillers_hs0=()):
                """S+exp+O for head pair p, query half qh, one head at a
                time.  fillers: step-closures drained at a per-slot rate
                across the block's 32 kc slots; fillers_hs0 must complete
                within the first head's 16 slots (e.g. v tiles this block's
                own O-matmuls read)."""
                qoff = qh * 1024
                nslots = 2 * NT
                slot = 0
                fillers_hs0 = list(fillers_hs0)

                def drain(last=False):
                    nonlocal slot
                    slot += 1
                    if fillers_hs0:
                        deadline = NT - 2
                        remaining = max(deadline - slot, 1)
                        rate = math.ceil(len(fillers_hs0) / remaining)
                        if slot >= deadline:
                            rate = len(fillers_hs0)
                        for _ in range(min(rate, len(fillers_hs0))):
                            fillers_hs0.pop(0)()
                        return
                    if last:
                        while fillers:
                            fillers.pop(0)()
                        return
                    remaining = nslots - slot
                    if remaining <= 0:
                        rate = len(fillers)
                    else:
                        rate = math.ceil(len(fillers) / remaining)
                    for _ in range(min(rate, len(fillers))):
                        fillers.pop(0)()

                for hs in range(2):
                    off = hs * 64
                    e_tiles = {}
                    oaccs = None

                    def o_step(kc):
                        for qc in range(2):
                            nc.tensor.matmul(
                                out=oaccs[qc],
                                lhsT=vaug[:, kc, 2 * p + hs, :],
                                rhs=e_tiles[kc][:, qc * 512 : (qc + 1) * 512],
                                start=(kc == 0),
                                stop=(kc == NT - 1),
                            )

                    for kc in range(NT):
                        stile = pbig.tile([128, 1024], F32, tag="big", name="s")
                        for qc in range(2):
                            nc.tensor.matmul(
                                out=stile[:, qc * 512 : (qc + 1) * 512],
                                lhsT=kt[p][
                                    off : off + 64, kc * 128 : (kc + 1) * 128
                                ],
                                rhs=qt[p][
                                    off : off + 64,
                                    qoff + qc * 512 : qoff + (qc + 1) * 512,
                                ],
                                start=True,
                                stop=True,
                            )
                        e = e2p.tile([128, 1024], BF16, tag="e2", name="e")
                        nc.scalar.activation(
                            out=e,
                            in_=stile,
                            func=mybir.ActivationFunctionType.Exp,
                            scale=SCALE,
                        )
                        e_tiles[kc] = e
                        if oaccs is None:
                            oaccs = [
                                poacc.tile([65, 512], F32, tag="oa", name="oacc")
                                for _ in range(2)
                            ]
                        if kc >= ODELAY:
                            o_step(kc - ODELAY)
                        drain(last=(hs == 1 and kc == NT - 1))
                    for kc in range(NT - ODELAY, NT):
                        o_step(kc)
                    # epilogue per query chunk: normalize rows by the
                    # denominator (psum row 64, staged through SBUF - the
                    # approx reciprocal misreads PSUM directly)
                    for qc in range(2):
                        lraw = small.tile([1, 512], F32, tag="lraw", name="lraw")
                        nc.vector.tensor_copy(out=lraw, in_=oaccs[qc][64:65, :])
                        lrow = small.tile([1, 512], F32, tag="lrow", name="lrow")
                        nc.vector.reciprocal_approx_fast(out=lrow, in_=lraw)
                        lb = small.tile([64, 512], F32, tag="lb", name="lb")
                        nc.gpsimd.partition_broadcast(lb, lrow)
                        nc.vector.tensor_mul(
                            out=attnt[p][
                                off : off + 64,
                                qoff + qc * 512 : qoff + (qc + 1) * 512,
                            ],
                            in0=oaccs[qc][0:64, :],
                            in1=lb,
                        )

            # ---------------- emission order ----------------------------
            # m-loop: stats(m) | transposes(m-1) | v(m-2), for m tiles 0-7;
            # qk(p0) qt-h0/kt-h0 interleaved once tokens 0-1023 are up.
            vq = {}
            mloop_fill = []
            for m in range(NT + 2):
                if m < NT:
                    vq[m] = ln_stats(m, nsplit=8 if m < 2 else 2)
                if 1 <= m <= NT:
                    ln_transpose(m - 1, vq.pop(m - 1))
                if 2 <= m < 10:
                    for step in v_steps(m - 2):
                        step()
                if m == 10:
                    mloop_fill += qk_quarter_steps(0, wq, qt[0], 0)
                if m == 12:
                    mloop_fill += qk_quarter_steps(0, wk, kt[0], 0)
                while mloop_fill and m >= 10:
                    mloop_fill.pop(0)()
            # kt(p0) half1 gates the first attention block: emit densely
            for step in qk_quarter_steps(0, wk, kt[0], 1):
                step()

            Q, K = 0, 1

            def quarters(p, wh, half):
                w_dram, dst = ((wq, qt[p]) if wh == Q else (wk, kt[p]))
                return qk_quarter_steps(p, w_dram, dst, half)

            # per-(qh, p) filler assignment; each entry must be complete
            # before the block that reads it starts (see gating comments)
            plan = {
                (0, 0): [(None, (1, K, 1)), (None, (1, Q, 0)), (None, (1, K, 0))],
                (0, 1): [(None, (2, K, 1)), (None, (2, Q, 0)), (None, (2, K, 0))],
                (0, 2): [(None, (3, K, 1)), (None, (3, Q, 0)), (None, (3, K, 0))],
                (0, 3): [(None, (0, Q, 1))],
                (1, 0): [(None, (1, Q, 1)), ("op", 0), ("op", 1)],
                (1, 1): [(None, (2, Q, 1)), ("op", 2), ("op", 3)],
                (1, 2): [(None, (3, Q, 1)), ("op", 4), ("op", 5)],
                (1, 3): [("op", 6), ("op", 7)],
            }

            for qh in range(2):
                for p in range(NP):
                    steps = []
                    for kind, arg in plan[(qh, p)]:
                        if kind == "op":
                            steps += outproj_steps(arg)
                        else:
                            steps += quarters(*arg)
                    hs0_steps = []
                    if (qh, p) == (0, 0):
                        for m in range(8, NT):
                            hs0_steps += v_steps(m)
                    attention_block(p, qh, steps, fillers_hs0=hs0_steps)

            # tail: second-half output projection, dense
            for m in range(8, 16):
                for step in outproj_steps(m):
                    step()

    nc.compile()
    return nc


def _get_nc():
    if "nc" not in _nc_cache:
        _nc_cache["nc"] = _build_nc()
    return _nc_cache["nc"]


def _make_in_maps(q, ln_gamma, ln_beta, W_q, W_kv, W_out):
    q = np.asarray(q, dtype=np.float32)
    g = np.asarray(ln_gamma, dtype=np.float32)
    beta = np.asarray(ln_beta, dtype=np.float32)
    W_q = np.asarray(W_q, dtype=np.float32)
    W_kv = np.asarray(W_kv, dtype=np.float32)
    W_out = np.asarray(W_out, dtype=np.float32)

    assert np.allclose(beta, 0.0, atol=1e-30), (
        "nonzero ln_beta not supported by this kernel build"
    )
    wq_full = (g[:, None] * W_q).astype(ml_dtypes.bfloat16)
    wk_full = (g[:, None] * W_kv[:, :E]).astype(ml_dtypes.bfloat16)
    wv_full = (g[:, None] * W_kv[:, E:]).astype(ml_dtypes.bfloat16)
    wo_full = W_out.astype(ml_dtypes.bfloat16)

    in_maps = []
    for c in range(NCORES):
        b, grp = c // 2, c % 2
        cols = slice(grp * EL, (grp + 1) * EL)
        in_maps.append(
            {
                "x": np.ascontiguousarray(q[b].astype(ml_dtypes.bfloat16)),
                "wq": np.ascontiguousarray(wq_full[:, cols]),
                "wk": np.ascontiguousarray(wk_full[:, cols]),
                "wv": np.ascontiguousarray(wv_full[:, cols]),
                "wo": np.ascontiguousarray(wo_full[cols, :]),
            }
        )
    return in_maps


def _gather(results):
    out = np.empty((B, N, D), dtype=np.float32)
    for b in range(B):
        out[b] = results[2 * b]["out"] + results[2 * b + 1]["out"]
    return out


def kernel(q, ln_gamma, ln_beta, W_q, W_kv, W_out):
    nc = _get_nc()
    in_maps = _make_in_maps(q, ln_gamma, ln_beta, W_q, W_kv, W_out)
    res = run_bass_kernel_spmd(nc, in_maps, core_ids=list(range(NCORES)))
    return _gather(res.results)


def kernel_traced(q, ln_gamma, ln_beta, W_q, W_kv, W_out):
    """Like kernel() but with NTFF profiling; returns (out, BassKernelResults)."""
    nc = _get_nc()
    in_maps = _make_in_maps(q, ln_gamma, ln_beta, W_q, W_kv, W_out)
    res = run_bass_kernel_spmd(nc, in_maps, core_ids=list(range(NCORES)), trace=True)
    return _gather(res.results), res
